# revision 5
# baseline (speedup 1.0000x reference)
"""AttentionBlock Trainium2 kernel (v3: fp8 DoubleRow for QK-proj and AV).

Data-parallel: one batch element per NeuronCore (8 cores, no collectives).

Per core, with xr = x[b] viewed as [C, S] (C=512 channels, S=1024 tokens):
    QT = wq^T @ xr + bq   -> [D, S]  (d on partitions; head h = rows 64h..64h+63)
    KT = wk^T @ xr + bk   -> [D, S]
    V  = xr^T @ wv + bv   -> [S, D]  (tokens on partitions)
    per head h: ET[j, i] = KT_h^T . QT_h            (keys j on psum partitions)
                E = exp(ET / sqrt(C) - G)           (G global offset, cancels in
                                                     the O'/Z ratio; keeps E in
                                                     e5m2 range: max e ~ 11)
                O'T[d, i] = sum_j V[j, d] E[j, i];  Z[i] = sum_j E[j, i]
                OT[d, i] = O'T[d, i] / Z[i]
    y = wo^T @ OT + bo + xr   -> [C, S]

dtypes / PE modes (measured: fp8 DoubleRow streams 2 fp8/cycle = 2x fp16 rate):
  - QK projection: DoubleRow fp8e4 (x8 [128,2j,2i,S], wq8/wk8 [128,2j,2i,512]),
    contraction 512 = 2 chunks x (128 partitions x 2 k-tiles). 8 matmuls/pair
    (was 16 fp16). psum f32 -> ACT Identity+bias copy to fp16 QT/KT.
  - energy: fp16, K=64 head pairs at partition offsets 0/64 (as v2). No fp8
    gain possible: at 64-contraction the moving stream is byte-bound either way.
  - AV: DoubleRow, et e5m2 x Vp e4m3 (mixed dtypes verified on HW). Key chunks
    handled in pairs: lhsT Vp8[:,p,h] = [128, 2, 128], rhs et pair tile
    [128, 2, 512]. 8 matmuls per (t,i) (was 16 fp16). The ones|V column split
    still lands Z on psum rows 0..63 (ones in e4m3 are exact).
  - V projection and final projection stay fp16: their quantization error goes
    straight to the output (no softmax averaging) and measured at ~1e-2 rel
    alone in emulation -- too close to the 2e-2 gate.

exp splits between ScalarE (exact table exp -> e5m2, scale=1/sqrt(C), bias=-G)
and DVE (Schraudolph e5m2 bit-trick: u8(x*A + B), ~9% per-element, cancels in
the softmax ratio). ATTN_DVE_EXP_UNITS picks which of the 8 key-chunk units per
(t, i) go to DVE; numerics emulation prefers mid units (3,4,5): rel ~9.8e-3.

Pipelining (kept from v2): energy pair -> exp -> AV deferred one pair; PSUM
8 banks: energy 2x2 (double-buffered), AV 1x2, QK/vproj/warm aux 1x2. The
previous half's norm flushes at each half start after the first exp is queued.
QK DoubleRow for pair t+1 is hoisted mid-way through t (now only 8 matmuls);
its psum->fp16 copies run on ACT (Identity + per-partition bias).
"""

import math
import os

import numpy as np

B = 8
C = 512
S = 1024  # 32*32 tokens
NH = 8
HD = 64
P = 128
CC = C // P  # 4 contraction chunks of 128
NI = 2  # S split into 2 chunks of 512 for matmul free dim
SC = S // P  # 8 key chunks of 128
NP = SC // 2  # 4 key-chunk pairs for the DoubleRow AV

G_OFF = float(os.environ.get("ATTN_G", "2.0"))
SCHRAUD_C = float(os.environ.get("ATTN_SCHRAUD_C", "0.5"))
N_WARM = int(os.environ.get("ATTN_WARM", "14"))
_DVE_UNITS = tuple(
    int(u) for u in os.environ.get("ATTN_DVE_EXP_UNITS", "3,4,5").split(",") if u != ""
)
# which engine copies QK psum->fp16: "act" or "dve"
QK_COPY = os.environ.get("ATTN_QK_COPY", "act")


def _emit(nc, tc, mybir, aps):
    import contextlib

    F32 = mybir.dt.float32
    F16 = mybir.dt.float16
    E4 = mybir.dt.float8e4
    E5 = mybir.dt.float8e5
    U8 = mybir.dt.uint8
    U16 = mybir.dt.uint16
    DR = mybir.MatmulPerfMode.DoubleRow
    MULT = mybir.AluOpType.mult
    ADD = mybir.AluOpType.add
    EXP = mybir.ActivationFunctionType.Exp
    IDENT = mybir.ActivationFunctionType.Identity
    softmax_scale = 1.0 / math.sqrt(C)
    ONE_F16 = 0x3C00
    ONE_E4 = 0x38  # 1.0 in e4m3

    # Schraudolph constants for e5m2 bit-pattern exp of (scale*x - G):
    # u8(x*scale*4/ln2 + (15*4 - c - G*4/ln2))
    SCH_A = softmax_scale * 4.0 / math.log(2.0)
    SCH_B = 15.0 * 4.0 - SCHRAUD_C - G_OFF * 4.0 / math.log(2.0)

    xb, x8, wq8, wk8, bq, bk, wv, bv, wo, bo, y = (
        aps[k]
        for k in ("xb", "x8", "wq8", "wk8", "bq", "bk", "wv", "bv", "wo", "bo", "y")
    )
    xb_r = xb.rearrange("(cc p) s -> p cc s", p=P)
    y_r = y.rearrange("(cc p) s -> p cc s", p=P)
    # DoubleRow layouts: [p, j(chunk), i(k-tile), ...]
    x8_r = x8.rearrange("(j i p) s -> p j i s", p=P, i=2)
    wq8_r = wq8.rearrange("(j i p) d -> p j i d", p=P, i=2)
    wk8_r = wk8.rearrange("(j i p) d -> p j i d", p=P, i=2)
    wv_r = wv.rearrange("(cc p) d -> p cc d", p=P)
    wo_r = wo.rearrange("(dc p) c -> p dc c", p=P)
    bq_r = bq.rearrange("(dc p) -> p dc", p=P)
    bk_r = bk.rearrange("(dc p) -> p dc", p=P)
    bo_r = bo.rearrange("(cc p) -> p cc", p=P)

    with contextlib.ExitStack() as ctx:
        singles = ctx.enter_context(tc.tile_pool(name="singles", bufs=1))
        qkpool = ctx.enter_context(tc.tile_pool(name="qk", bufs=2))
        etpool = ctx.enter_context(tc.tile_pool(name="et", bufs=4))
        rbpool = ctx.enter_context(tc.tile_pool(name="rb", bufs=4))
        tmppool = ctx.enter_context(tc.tile_pool(name="tmp", bufs=4))
        # PSUM (8 banks): energy 2 tiles x 2 banks, AV 1 x 2, aux 1 x 2.
        pse = ctx.enter_context(tc.tile_pool(name="pse", bufs=2, space="PSUM"))
        psav = ctx.enter_context(tc.tile_pool(name="psav", bufs=1, space="PSUM"))
        psaux = ctx.enter_context(tc.tile_pool(name="psaux", bufs=1, space="PSUM"))

        # ---- input DMAs, spread across queues, first-needed first ----
        xb_sb = singles.tile([P, CC, S], F16)
        x8_sb = singles.tile([P, 2, 2, S], E4)
        wq8_sb = singles.tile([P, 2, 2, C], E4)
        wk8_sb = singles.tile([P, 2, 2, C], E4)
        bq_sb = singles.tile([P, CC], F32)
        bk_sb = singles.tile([P, CC], F32)
        bo_sb = singles.tile([P, CC], F32)
        bv_sb = singles.tile([1, C], F16)
        wv_sb = singles.tile([P, CC, C], F16)
        wo_sb = singles.tile([P, CC, C], F16)

        # x8 gates the t=0 QK DoubleRow matmuls; wq8/wk8 are small (256KB each).
        nc.sync.dma_start(out=x8_sb, in_=x8_r)
        nc.scalar.dma_start(out=wq8_sb, in_=wq8_r)
        nc.scalar.dma_start(out=wk8_sb, in_=wk8_r)
        nc.scalar.dma_start(out=bq_sb, in_=bq_r)
        nc.scalar.dma_start(out=bk_sb, in_=bk_r)
        nc.scalar.dma_start(out=wv_sb, in_=wv_r)
        # xb fp16 feeds the V projection (t=0 mid-stream) and the residual.
        for cc in range(CC - 1):
            nc.sync.dma_start(out=xb_sb[:, cc], in_=xb_r[:, cc])
        nc.gpsimd.dma_start(out=xb_sb[:, 3], in_=xb_r[:, 3])
        nc.gpsimd.dma_start(out=bv_sb, in_=bv[None, :])
        nc.gpsimd.dma_start(out=bo_sb, in_=bo_r)
        nc.gpsimd.dma_start(out=wo_sb, in_=wo_r)

        # V' stationary for the DoubleRow AV: per (pair p, head h, slot s):
        # 128 cols = [ones(64) | V(64)] in e4m3. ones memset on GpSimd (SBUF
        # ok; GpSimd may not touch PSUM). The V copies fill cols 64..127.
        Vp8 = singles.tile([P, NP, NH, 2, P], E4)
        for pp in range(NP):
            nc.gpsimd.memset(Vp8[:, pp, :, :, 0:64].bitcast(U8), ONE_E4)
        bv_rep = singles.tile([P, C], F16)
        nc.gpsimd.partition_broadcast(bv_rep, bv_sb, channels=P)

        # per-partition constant -G for the ACT exp bias
        negg_sb = singles.tile([P, 1], F32)
        nc.vector.memset(negg_sb, -G_OFF)

        # PE warm-up on zeros while input DMAs land
        warm = singles.tile([P, 512], F16)
        nc.vector.memset(warm.bitcast(U16), 0)
        ps_w = psaux.tile([P, 2, 512], F32, tag="aux")
        for _ in range(N_WARM):
            nc.tensor.matmul(ps_w[:, 0], warm[:, 0:128], warm)

        OTs = [singles.tile([P, S], F16, tag=f"ot{t}", name=f"ot{t}") for t in range(CC)]

        def emit_v_projection_chunk(sc):
            # V[s, d] = xr^T @ wv + bv for one token chunk (fp16); output cast
            # to e4m3 into the Vp8 value columns.
            ps_v = pse.tile([P, 2, 512], F32, tag="e")
            for cc in range(CC):
                nc.tensor.matmul(
                    ps_v[:, 0],
                    xb_sb[:, cc, sc * P : (sc + 1) * P],
                    wv_sb[:, cc],
                    start=(cc == 0),
                    stop=(cc == CC - 1),
                )
            psv_r = ps_v[:, 0].rearrange("p (h d) -> p h d", h=NH)
            bv_r2 = bv_rep.rearrange("p (h d) -> p h d", h=NH)
            nc.vector.tensor_tensor(
                Vp8[:, sc // 2, :, sc % 2, 64:128], psv_r, bv_r2, ADD
            )

        pending_norm = [None]
        pending_av = []

        def flush_av(depth=0):
            while len(pending_av) > depth:
                pending_av.pop(0)()

        def flush_norm():
            if pending_norm[0] is not None:
                pending_norm[0]()
                pending_norm[0] = None

        qk_tiles = {}

        def qk_copy(dst, src, bias_ap):
            if QK_COPY == "act":
                nc.scalar.activation(out=dst, in_=src, func=IDENT, bias=bias_ap, scale=1.0)
            else:
                nc.vector.tensor_scalar_add(dst, src, bias_ap)

        def make_qk_steps(tn):
            # Hoisted DoubleRow QK for head-pair tn, in three steps spread
            # across the stream: i0 matmuls; i0 copies + i1 matmuls; i1 copies.
            qt = qkpool.tile([P, S], F16, tag="qt", name=f"qt{tn}")
            kt = qkpool.tile([P, S], F16, tag="kt", name=f"kt{tn}")
            qk_tiles[tn] = (qt, kt)
            ps_list = []
            dsl = slice(tn * P, (tn + 1) * P)

            def mms(i):
                sl = slice(i * 512, (i + 1) * 512)
                ps_p = psaux.tile([P, 2, 512], F32, tag="aux")
                ps_list.append(ps_p)
                for j in range(2):
                    nc.tensor.matmul(
                        ps_p[:, 0], wq8_sb[:, j, :, dsl], x8_sb[:, j, :, sl],
                        start=(j == 0), stop=(j == 1), perf_mode=DR,
                    )
                    nc.tensor.matmul(
                        ps_p[:, 1], wk8_sb[:, j, :, dsl], x8_sb[:, j, :, sl],
                        start=(j == 0), stop=(j == 1), perf_mode=DR,
                    )

            def bias(i):
                sl = slice(i * 512, (i + 1) * 512)
                qk_copy(qt[:, sl], ps_list[i][:, 0], bq_sb[:, tn : tn + 1])
                qk_copy(kt[:, sl], ps_list[i][:, 1], bk_sb[:, tn : tn + 1])

            return [
                lambda: mms(0),
                lambda: (bias(0), mms(1)),
                lambda: bias(1),
            ]

        def emit_qk(t):
            for step in make_qk_steps(t):
                step()

        # ---- per head-pair t ----
        emit_qk(0)
        for t in range(CC):
            qt, kt = qk_tiles.pop(t)
            h0, h1 = 2 * t, 2 * t + 1
            for i in range(NI):
                sl = slice(i * 512, (i + 1) * 512)
                ps_av = None
                et_pair = None
                for jc in range(SC):
                    ih = 1 if t == 0 else 0
                    if i == ih and t < CC - 1:
                        if jc == 2:
                            qk_steps = make_qk_steps(t + 1)
                            qk_steps[0]()
                        elif jc == 5:
                            qk_steps[1]()
                        elif jc == 7:
                            qk_steps[2]()
                    k0 = kt[0:64, jc * P : (jc + 1) * P]
                    k1 = kt[64:128, jc * P : (jc + 1) * P]
                    ps_e = pse.tile([P, 2, 512], F32, tag="e")  # head-major
                    nc.tensor.matmul(ps_e[:, 0], k0, qt[0:64, sl])
                    nc.tensor.matmul(ps_e[:, 1], k1, qt[64:128, sl])
                    if t == 0 and i == 0:
                        # vproj staggered so the wv DMA never blocks the early
                        # exp stream; each chunk lands before its deferred AV
                        for sc in {2: (0,), 3: (1,), 4: (2,), 5: (3,),
                                   6: (4, 5), 7: (6, 7)}.get(jc, ()):
                            emit_v_projection_chunk(sc)
                    if jc % 2 == 0:
                        et_pair = etpool.tile([P, 2, 2, 512], E5, tag="et")
                    et_out = et_pair[:, jc % 2]
                    if jc in _DVE_UNITS:
                        # Schraudolph e5m2 bit-pattern exp on DVE
                        nc.vector.tensor_scalar(
                            et_out.bitcast(U8), ps_e, SCH_A, SCH_B, MULT, ADD
                        )
                    else:
                        nc.scalar.activation(
                            out=et_out, in_=ps_e, func=EXP,
                            scale=softmax_scale, bias=negg_sb[:, 0:1],
                        )
                    if jc == 0:
                        # Drain the previous half's trailing AVs and emit its
                        # norm only AFTER this half's first energy+exp are in
                        # the queues (ScalarE crosses the seam with zero idle).
                        flush_av()
                        flush_norm()
                        ps_av = psav.tile([P, 2, 512], F32, tag="av")
                    if jc % 2 == 1:
                        # AV for this pair, deferred one pair so the in-order
                        # PE stream never waits on this pair's exp
                        flush_av(depth=1)
                        pp = jc // 2

                        def av(ps_av=ps_av, pp=pp, et=et_pair, h0=h0, h1=h1):
                            nc.tensor.matmul(
                                ps_av[:, 0], Vp8[:, pp, h0], et[:, :, 0],
                                start=(pp == 0), stop=(pp == NP - 1),
                                perf_mode=DR,
                            )
                            nc.tensor.matmul(
                                ps_av[:, 1], Vp8[:, pp, h1], et[:, :, 1],
                                start=(pp == 0), stop=(pp == NP - 1),
                                perf_mode=DR,
                            )

                        pending_av.append(av)

                def norm(t=t, sl=sl, ps_av=ps_av):
                    # Z replicated on psum rows 0..63 (ones half of V'); O' on
                    # rows 64..127.
                    rb = rbpool.tile([64, 2, 512], F32, tag="rb")
                    nc.vector.reciprocal_approx_fast(out=rb, in_=ps_av[0:64])
                    nc.vector.tensor_tensor(
                        OTs[t][0:64, sl], ps_av[64:128, 0], rb[:, 0], MULT
                    )
                    nc.vector.tensor_tensor(
                        OTs[t][64:128, sl], ps_av[64:128, 1], rb[:, 1], MULT
                    )

                pending_norm[0] = norm

        # ---- final projection + bias + residual (fp16, as v2) ----
        flush_av()
        flush_norm()
        ps_fs = [
            pse.tile([P, 2, 512], F32, tag="e", name="psf0"),
            pse.tile([P, 2, 512], F32, tag="e", name="psf1"),
            psaux.tile([P, 2, 512], F32, tag="aux", name="psf2"),
            psav.tile([P, 2, 512], F32, tag="av", name="psf3"),
        ]

        def fmm(dc, cc, start, stop):
            wo_sl = wo_sb[:, dc, cc * P : (cc + 1) * P]
            for i in range(NI):
                sl = slice(i * 512, (i + 1) * 512)
                nc.tensor.matmul(
                    ps_fs[cc][:, i], wo_sl, OTs[dc][:, sl], start=start, stop=stop,
                )

        for dc in range(CC - 1):
            for cc in range(CC - 1):
                fmm(dc, cc, dc == 0, False)
        for dc in range(CC - 1):
            fmm(dc, 3, dc == 0, False)
        out_q = [nc.sync, nc.scalar, nc.gpsimd]
        for cc in range(CC):
            fmm(CC - 1, cc, False, True)
            for i in range(NI):
                sl = slice(i * 512, (i + 1) * 512)
                tmp = tmppool.tile([P, 512], F16, tag="tmp")
                nc.vector.scalar_tensor_tensor(
                    out=tmp,
                    in0=ps_fs[cc][:, i],
                    scalar=bo_sb[:, cc : cc + 1],
                    in1=xb_sb[:, cc, sl],
                    op0=ADD,
                    op1=ADD,
                )
                out_q[(2 * cc + i) % 3].dma_start(out=y_r[:, cc, sl], in_=tmp)


_NC_CACHE = {}


def _build():
    key = (G_OFF, SCHRAUD_C, N_WARM, _DVE_UNITS, QK_COPY)
    if key in _NC_CACHE:
        return _NC_CACHE[key]
    import concourse.bacc as bacc
    import concourse.mybir as mybir
    import concourse.tile as tile

    F32 = mybir.dt.float32
    F16 = mybir.dt.float16
    E4 = mybir.dt.float8e4
    nc = bacc.Bacc("TRN2", target_bir_lowering=False, debug=False)
    aps = {}
    aps["xb"] = nc.dram_tensor("xb", (C, S), F16, kind="ExternalInput").ap()
    aps["x8"] = nc.dram_tensor("x8", (C, S), E4, kind="ExternalInput").ap()
    aps["wq8"] = nc.dram_tensor("wq8", (C, C), E4, kind="ExternalInput").ap()
    aps["wk8"] = nc.dram_tensor("wk8", (C, C), E4, kind="ExternalInput").ap()
    aps["wv"] = nc.dram_tensor("wv", (C, C), F16, kind="ExternalInput").ap()
    aps["wo"] = nc.dram_tensor("wo", (C, C), F16, kind="ExternalInput").ap()
    for name in ("bq", "bk", "bo"):
        aps[name] = nc.dram_tensor(name, (C,), F32, kind="ExternalInput").ap()
    aps["bv"] = nc.dram_tensor("bv", (C,), F16, kind="ExternalInput").ap()
    aps["y"] = nc.dram_tensor("y", (C, S), F16, kind="ExternalOutput").ap()
    with tile.TileContext(nc) as tc:
        _emit(nc, tc, mybir, aps)
    nc.compile()
    _NC_CACHE[key] = nc
    return nc


def prepare_in_maps(x, wq, bq, wk, bk, wv, bv, wo, bo):
    """Host-side prep: cast to f16/e4m3, shard x per core."""
    import ml_dtypes

    E4 = ml_dtypes.float8_e4m3
    x = np.asarray(x, dtype=np.float32).reshape(B, C, S)
    weights = {
        "wq8": np.ascontiguousarray(np.asarray(wq, dtype=np.float32).astype(E4)),
        "bq": np.ascontiguousarray(np.asarray(bq, dtype=np.float32)),
        "wk8": np.ascontiguousarray(np.asarray(wk, dtype=np.float32).astype(E4)),
        "bk": np.ascontiguousarray(np.asarray(bk, dtype=np.float32)),
        "wv": np.ascontiguousarray(np.asarray(wv, dtype=np.float16)),
        "bv": np.ascontiguousarray(np.asarray(bv, dtype=np.float16)),
        "wo": np.ascontiguousarray(np.asarray(wo, dtype=np.float16)),
        "bo": np.ascontiguousarray(np.asarray(bo, dtype=np.float32)),
    }
    return [
        {
            "xb": np.ascontiguousarray(x[b].astype(np.float16)),
            "x8": np.ascontiguousarray(x[b].astype(E4)),
            **weights,
        }
        for b in range(B)
    ]


def kernel(x, wq, bq, wk, bk, wv, bv, wo, bo):
    from concourse import bass_utils

    nc = _build()
    in_maps = prepare_in_maps(x, wq, bq, wk, bk, wv, bv, wo, bo)
    res = bass_utils.run_bass_kernel_spmd(nc, in_maps, core_ids=list(range(B)))
    out = np.stack([r["y"].astype(np.float32) for r in res.results])
    return out.reshape(B, C, 32, 32)
